# revision 30
# baseline (speedup 1.0000x reference)
"""DepthwiseXCorr (SiamRPN-style depthwise cross-correlation head) on 8 trn2 cores.

Data-parallel over batch: B=128 -> 16 samples per core. Per sample:
  branch(x) = BN2(pw1x1(ReLU6(BN1(dw3x3(x)))))   for kernel (7x7) and search (31x31)
  out = per-channel xcorr(search_feat 29x29, kernel_feat 5x5) -> 25x25

Final design (~322us vs 427us v2 baseline; PE/ACT/DVE all ~90% busy):
  - search dw conv: PE diag matmuls in float32r directly on the raw f32
    input (1 cyc/row at N>=256; innermost AP dim must be EVEN, so windows
    are 30 wide on a flat padded tile, garbage last column skipped at the
    ACT evict).  No input conversion pass at all.
  - kernel branch: fully batched across the 16 samples (one 480-col pass
    per tap / pw block instead of 16 tiny 25-col passes).
  - xcorr per 25-tap block: 14-15 "D" taps run as fused diag-matmuls on
    PE (product+accumulate in one 500+125 PSUM pair, ~370ns); their diag
    matrices are built in ONE DVE broadcast-tt (ident x K2-bcast, 1x mode
    ~2us per block).  5-6 "B" taps are ACT Copy-scale products (~950ns)
    and 4-5 "W" taps are DVE ts products (~670ns); B/W products fold into
    an fp16 accumulator via DVE tt-adds, merged into PSUM by two identity
    matmuls; ACT evicts f32, DMA out.
  - Measured dead ends: scalar_tensor_tensor on [625] runs ~4.2us (never
    use); GPSIMD elementwise ~1.4-9us AND its SBUF port contends with
    DVE (poisons DVE to 2-4x); fp8 DoubleRow dw-conv gives rel err 1.5e-2
    (too close to the 2e-2 gate); deeper tile pools (bufs=3) regress.
"""

import numpy as np

import concourse.bass as bass
import concourse.mybir as mybir
from concourse.tile import TileContext
from concourse.bass_utils import run_bass_kernel_spmd

F32 = mybir.dt.float32
F32R = mybir.dt.float32r
F16 = mybir.dt.float16
AF = mybir.ActivationFunctionType
OP = mybir.AluOpType

B, C, KH, SH, KK = 128, 256, 7, 31, 3
N_CORES = 8
BPC = B // N_CORES          # samples per core
G = C // 128                # channel blocks
EPS = 1e-5

HO_K, HO_S, HO_X = KH - 2, SH - 2, 25   # 5, 29, 25

# f32 bias param columns: [bdk G | bds G | bpk G | bps G]
O_BDK, O_BDS, O_BPK, O_BPS = 0, G, 2 * G, 3 * G
P_F32 = 4 * G
# fp16 param columns: [Ws G*G*128 | Wk G*G*128 | ident 128]
O_WS, O_WK, O_ID = 0, G * G * 128, 2 * G * G * 128
P_F16 = 2 * G * G * 128 + 128

# xcorr tap routing per 25-tap block (measured costs, see microbench):
#   D: fused diag-matmul on PE (376ns/pair; diags built ~free on DVE)
#   B: ACT product (950ns) + DVE tt-add (590ns)
#   W: DVE ts product (670ns) + DVE tt-add
# Split tuned so PE ~ ACT ~ DVE ~ 19-20us/sample.
_ALL = [(u, v) for u in range(5) for v in range(5)]
# per-block counts: (D fused-diag on PE, B ACT-product, W DVE-product);
# D-taps must be a prefix (their diags are built from contiguous K2 cols).
ND0, NB0 = 13, 7      # g=0: W0 = 25-13-7 = 5
ND1, NB1 = 14, 6      # g=1: W1 = 5
# number of tt-adds per block routed to GPSIMD instead of DVE
GPS_ADDS = 3

_cache: dict = {}

LAST_RESULTS = None         # stash for test harness (exec_time_ns etc.)


def _fold_branch(dw_w, bn1, pw_w, pw_b, bn2):
    """Fold eval-mode BN params into conv weights/biases (host, numpy fp32)."""
    g1, b1, m1, v1 = bn1[0], bn1[1], bn1[2], bn1[3]
    inv1 = g1 / np.sqrt(v1 + EPS)
    shift1 = b1 - m1 * inv1
    dw = (dw_w[:, 0] * inv1[:, None, None]).reshape(C, 9).astype(np.float32)

    g2, b2, m2, v2 = bn2[0], bn2[1], bn2[2], bn2[3]
    inv2 = g2 / np.sqrt(v2 + EPS)
    shift2 = b2 - m2 * inv2
    W = (pw_w[:, :, 0, 0] * inv2[:, None]).astype(np.float32)   # (co, ci)
    bias2 = (pw_b * inv2 + shift2).astype(np.float32)

    lhsT = np.zeros((G, G, 128, 128), np.float32)
    for gi in range(G):
        for go in range(G):
            lhsT[gi, go] = W[go * 128:(go + 1) * 128, gi * 128:(gi + 1) * 128].T
    dw_blk = dw.reshape(G, 128, 9)
    b1_blk = shift1.astype(np.float32).reshape(G, 128, 1)
    b2_blk = bias2.reshape(G, 128, 1)
    return dw_blk, b1_blk, lhsT, b2_blk


def _split_waits(nc, keep=1):
    """This container's walrus accepts only one sync-wait per instruction."""
    import bass_rust

    n = 0
    for bb in nc.m.functions[0].blocks:
        out = []
        for ins in bb.instructions:
            si = ins.sync_info
            if si is not None and len(si.on_wait) > keep:
                waits = list(si.on_wait)
                for w in waits[:-keep]:
                    n += 1
                    ev = mybir.InstEventSemaphore(
                        name=f"antsplitw_{n}", ins=[], outs=[])
                    ev.engine = ins.engine
                    ev.sync_info = bass_rust.SyncInfo(on_wait=[w], on_update=[])
                    out.append(ev)
                ins.sync_info = bass_rust.SyncInfo(
                    on_wait=waits[-keep:], on_update=list(si.on_update))
            out.append(ins)
        bb.instructions = out
    return n


def _build_nc():
    nc = bass.Bass()

    kern_h = nc.declare_dram_parameter("kern_in", [C, BPC, 64], F32R, isOutput=False)
    srch_h = nc.declare_dram_parameter("srch_in", [BPC, C, SH, SH], F32R, isOutput=False)
    prm_h = nc.declare_dram_parameter("params", [128, P_F32], F32, isOutput=False)
    sdiag_h = nc.declare_dram_parameter("sdiag", [128, G * 9 * 128], F32R, isOutput=False)
    kdiag_h = nc.declare_dram_parameter("kdiag", [128, G * 9 * 128], F32R, isOutput=False)
    prh_h = nc.declare_dram_parameter("params16", [128, P_F16], F16, isOutput=False)
    out_h = nc.declare_dram_parameter("out", [BPC, C, 625], F32, isOutput=True)

    with TileContext(nc) as tc:
        with (
            tc.tile_pool(name="const", bufs=1) as cpool,
            tc.tile_pool(name="kbatch", bufs=1) as kbpool,
            tc.tile_pool(name="sio", bufs=2) as spool,
            tc.tile_pool(name="feat", bufs=2) as fpool,
            tc.tile_pool(name="prod", bufs=2) as prpool,
            tc.tile_pool(name="xout", bufs=2) as xpool,
            tc.tile_pool(name="psdw", bufs=1, space="PSUM") as pdw,
            tc.tile_pool(name="pspw", bufs=1, space="PSUM") as ppw,
            tc.tile_pool(name="psx", bufs=2, space="PSUM") as px,
        ):
            # PE warm-up: the HAM clock gate keeps PE at 1.2GHz until ~3.4us
            # of sustained activity.  Burn that in with dummy matmuls during
            # the startup-DMA window (PE is idle then anyway) so real work
            # runs at 2.4GHz from the start.
            warm = cpool.tile([128, 512], F16)
            nc.gpsimd.memset(warm[:], 0.0)
            wps = pdw.tile([128, 480], F32, name="dA")
            for i in range(8):
                nc.tensor.matmul(wps[:], warm[:, 0:128], warm[:, 0:480],
                                 start=(i == 0), stop=(i == 7))

            # startup DMAs spread across HWDGE rings (sync/scalar/vector
            # each have their own FIFO) so the kernel-batch matmuls and the
            # first dw conv unblock as early as possible.
            kdiag = cpool.tile([128, G * 9 * 128], F32R)
            nc.sync.dma_start(out=kdiag[:], in_=kdiag_h[:])
            sdiag = cpool.tile([128, G * 9 * 128], F32R)
            nc.scalar.dma_start(out=sdiag[:], in_=sdiag_h[:])
            prm = cpool.tile([128, P_F32], F32)
            nc.scalar.dma_start(out=prm[:], in_=prm_h[:])
            prh = cpool.tile([128, P_F16], F16)
            nc.scalar.dma_start(out=prh[:], in_=prh_h[:])

            def _b(base, g):          # f32 bias col [128,1]
                return prm[:, base + g:base + g + 1]

            def _sdiag(g, t):         # f32r diag block [128,128]
                o = (g * 9 + t) * 128
                return sdiag[:, o:o + 128]

            def _kdiag(g, t):
                o = (g * 9 + t) * 128
                return kdiag[:, o:o + 128]

            def _ws(gi, go):
                o = O_WS + (gi * G + go) * 128
                return prh[:, o:o + 128]

            def _wk(gi, go):
                o = O_WK + (gi * G + go) * 128
                return prh[:, o:o + 128]

            ident = prh[:, O_ID:O_ID + 128]

            # ---- kernel branch, batched across all BPC samples ----
            # K2_all[:, g*BPC*25 + b*25 + t] = kernel feature (go-block g,
            # sample b, tap t), f32.
            K2 = cpool.tile([128, G * BPC * 25], F32)
            hk = []
            for g in range(G):
                xk = kbpool.tile([128, BPC, 8, 8], F32R, name=f"xk{g}")
                nc.sync.dma_start(
                    out=xk[:],
                    in_=kern_h[128 * g:128 * (g + 1)].rearrange(
                        "c b (u v) -> c b u v", u=8))
                # f32r matmuls need an even innermost dim: 6-wide windows
                # (the 6th column is garbage, skipped at evict).  PSUM tile
                # "dA" is sized 480 f32 and shared with the search dw conv.
                kbt = pdw.tile([128, 480], F32, name="dA")
                kbv = kbt[:].rearrange("p (b u v) -> p b u v", b=BPC, u=HO_K)
                for t in range(9):
                    u, v = t // 3, t % 3
                    nc.tensor.matmul(
                        kbv, _kdiag(g, t),
                        xk[:, :, u:u + HO_K, v:v + 6],
                        start=(t == 0), stop=(t == 8))
                h = kbpool.tile([128, BPC, HO_K, HO_K], F16, name=f"hk{g}")
                nc.scalar.activation(
                    h[:], kbv[:, :, :, 0:HO_K], AF.Relu,
                    bias=_b(O_BDK, g), scale=1.0)
                hk.append(h)
            for go in range(G):
                kpt = ppw.tile([128, 17, HO_S], F32, name="pA")
                kps = kpt[:].rearrange("p a c -> p (a c)")[:, 0:BPC * 25]
                for gi in range(G):
                    nc.tensor.matmul(kps, _wk(gi, go), hk[gi][:],
                                     start=(gi == 0), stop=(gi == G - 1))
                nc.scalar.activation(
                    K2[:, go * BPC * 25:(go + 1) * BPC * 25], kps,
                    AF.Identity, bias=_b(O_BPK, go), scale=1.0)

            def kcol(b, g, t):
                o = g * BPC * 25 + b * 25 + t
                return K2[:, o:o + 1]

            # ---- search branch (per sample) ----
            def emit_front_dw(b):
                S2 = []
                for g in range(G):
                    # flat tile (pad 3) so 30-wide windows can run off the
                    # row ends; the 30th psum column is garbage, skipped at
                    # evict.  f32r matmuls need an even innermost dim.
                    xs = spool.tile([128, SH * SH + 3], F32R, name=f"xs{g}")
                    nc.sync.dma_start(out=xs[:, 0:SH * SH],
                                      in_=srch_h[b, 128 * g:128 * (g + 1)])

                    def dwin(u, v, r0, nr):
                        o = (u + r0) * SH + v
                        return xs[:, o:o + nr * SH].rearrange(
                            "p (a c) -> p a c", a=nr)[:, :, 0:30]

                    dAt = pdw.tile([128, 480], F32, name="dA")
                    dA = dAt[:, 0:450].rearrange("p (a c) -> p a c", a=15)
                    dB = pdw.tile([128, 14, 30], F32, name="dB")
                    for t in range(9):
                        u, v = t // 3, t % 3
                        nc.tensor.matmul(dA, _sdiag(g, t), dwin(u, v, 0, 15),
                                         start=(t == 0), stop=(t == 8))
                        nc.tensor.matmul(dB[:], _sdiag(g, t), dwin(u, v, 15, 14),
                                         start=(t == 0), stop=(t == 8))
                    h = fpool.tile([128, HO_S, 32], F16, name=f"hs{g}")
                    nc.scalar.activation(h[:, 0:15, 0:HO_S], dA[:, :, 0:HO_S],
                                         AF.Relu, bias=_b(O_BDS, g), scale=1.0)
                    nc.scalar.activation(h[:, 15:29, 0:HO_S], dB[:, :, 0:HO_S],
                                         AF.Relu, bias=_b(O_BDS, g), scale=1.0)
                    S2.append(h)   # post-relu6 dw features per block
                return S2

            def emit_front_pw(b, hs):
                S2 = []
                for go in range(G):
                    pA = ppw.tile([128, 17, HO_S], F32, name="pA")
                    pB = ppw.tile([128, 12, HO_S], F32, name="pB")
                    for gi in range(G):
                        nc.tensor.matmul(pA[:], _ws(gi, go),
                                         hs[gi][:, 0:17, 0:HO_S],
                                         start=(gi == 0), stop=(gi == G - 1))
                        nc.tensor.matmul(pB[:], _ws(gi, go),
                                         hs[gi][:, 17:29, 0:HO_S],
                                         start=(gi == 0), stop=(gi == G - 1))
                    s2 = fpool.tile([128, HO_S, 32], F16, name=f"s2_{go}")
                    nc.scalar.activation(s2[:, 0:17, 0:HO_S], pA[:], AF.Identity,
                                         bias=_b(O_BPS, go), scale=1.0)
                    nc.scalar.activation(s2[:, 17:29, 0:HO_S], pB[:], AF.Identity,
                                         bias=_b(O_BPS, go), scale=1.0)
                    S2.append(s2)
                return S2

            def emit_xcorr(b, S2):
                for g in range(G):
                    nd = ND0 if g == 0 else ND1
                    nb = NB0 if g == 0 else NB1
                    d_taps = _ALL[:nd]
                    b_taps = _ALL[nd:nd + nb]
                    w_taps = _ALL[nd + nb:]

                    def win(u, v):
                        return S2[g][:, u:u + 25, v:v + 25]

                    def win2(u, v):      # split for the PE pair (500+125)
                        return (S2[g][:, u:u + 20, v:v + 25],
                                S2[g][:, u + 20:u + 25, v:v + 25])

                    # diag matrices for the D-taps in one broadcast-tt (DVE)
                    dall = prpool.tile([128, nd, 128], F16, name=f"dall{g}")
                    kbd = K2[:, g * BPC * 25 + b * 25:
                             g * BPC * 25 + b * 25 + nd]
                    itile = ident.rearrange("p (x c) -> p x c", x=1
                                            ).broadcast_to([128, nd, 128])
                    kbc = kbd.rearrange("p (t x) -> p t x", x=1
                                        ).broadcast_to([128, nd, 128])
                    nc.vector.tensor_tensor(dall[:], itile, kbc, OP.mult)

                    # B products (ACT) and W products (DVE ts); the first W
                    # product initializes the DVE accumulator, the first two
                    # B products initialize the GPSIMD accumulator.
                    acc = xpool.tile([128, 625], F16, name=f"xacc{g}")
                    accv = acc[:].rearrange("p (a c) -> p a c", a=25)
                    u0, v0 = w_taps[0]
                    nc.vector.tensor_scalar(accv, win(u0, v0),
                                            kcol(b, g, u0 * 5 + v0), None,
                                            OP.mult)
                    bprods = []
                    for slot, (u, v) in enumerate(b_taps):
                        p = prpool.tile([128, 625], F16, name=f"pr{g}_{slot}")
                        nc.scalar.activation(
                            p[:].rearrange("p (a c) -> p a c", a=25),
                            win(u, v), AF.Copy, bias=0.0,
                            scale=kcol(b, g, u * 5 + v))
                        bprods.append(p)
                    wprods = []
                    for slot, (u, v) in enumerate(w_taps[1:]):
                        p = prpool.tile([128, 625], F16, name=f"pw{g}_{slot}")
                        nc.vector.tensor_scalar(
                            p[:].rearrange("p (a c) -> p a c", a=25),
                            win(u, v), kcol(b, g, u * 5 + v), None, OP.mult)
                        wprods.append(p)
                    for p in bprods + wprods:
                        nc.vector.tensor_tensor(acc[:], p[:], acc[:], OP.add)

                    # PE: fused diag-matmul taps accumulate into PSUM
                    xA = px.tile([128, 500], F32, name="xA")
                    xB = px.tile([128, 128], F32, name="xB")
                    xAv = xA[:].rearrange("p (a c) -> p a c", a=20)
                    xBv = xB[:, 0:125].rearrange("p (a c) -> p a c", a=5)
                    for i, (u, v) in enumerate(d_taps):
                        st = (i == 0)
                        wa, wb = win2(u, v)
                        dg = dall[:, u * 5 + v]
                        nc.tensor.matmul(xAv, dg, wa, start=st, stop=False)
                        nc.tensor.matmul(xBv, dg, wb, start=st, stop=False)
                    # fold the fp16 side-accumulator into PSUM
                    nc.tensor.matmul(xA[:], ident, acc[:, 0:500],
                                     start=False, stop=True)
                    nc.tensor.matmul(xB[:, 0:125], ident, acc[:, 500:625],
                                     start=False, stop=True)

                    # ACT evicts PSUM -> SBUF f32, then DMA
                    of = xpool.tile([128, 625], F32, name=f"of{g}")
                    nc.scalar.activation(of[:, 0:500], xA[:], AF.Copy,
                                         bias=0.0, scale=1.0)
                    nc.scalar.activation(of[:, 500:625], xB[:, 0:125],
                                         AF.Copy, bias=0.0, scale=1.0)
                    nc.sync.dma_start(out=out_h[b, 128 * g:128 * (g + 1)],
                                      in_=of[:])

            # software pipeline: dw(b) [PE] first, then xcorr(b-1) whose
            # D-accs keep PE busy while ACT evicts dw(b), then pw(b) [PE].
            # Queues are in-order per engine: placing pw(b) after the ready
            # xcorr work prevents its dw-evict dependency from head-blocking
            # the PE queue.
            pend = None
            for b in range(BPC):
                hs = emit_front_dw(b)
                if pend is not None:
                    emit_xcorr(b - 1, pend)
                pend = emit_front_pw(b, hs)
            emit_xcorr(BPC - 1, pend)
    _split_waits(nc)
    return nc


def kernel(kernel, search, k_dw_w, k_bn1, k_pw_w, k_pw_b, k_bn2,
           s_dw_w, s_bn1, s_pw_w, s_pw_b, s_bn2):
    global LAST_RESULTS
    kdw, kb1, kpw, kb2 = _fold_branch(np.asarray(k_dw_w), np.asarray(k_bn1),
                                      np.asarray(k_pw_w), np.asarray(k_pw_b),
                                      np.asarray(k_bn2))
    sdw, sb1, spw, sb2 = _fold_branch(np.asarray(s_dw_w), np.asarray(s_bn1),
                                      np.asarray(s_pw_w), np.asarray(s_pw_b),
                                      np.asarray(s_bn2))
    kern = np.zeros((B, C, 8, 8), np.float32)
    kern[:, :, 0:KH, 0:KH] = np.asarray(kernel, np.float32)
    kern = kern.reshape(B, C, 64)
    srch = np.ascontiguousarray(np.asarray(search, np.float32))

    if "nc" not in _cache:
        _cache["nc"] = _build_nc()
    nc = _cache["nc"]

    prm = np.zeros((128, P_F32), np.float32)
    prm[:, O_BDK:O_BDK + G] = kb1.transpose(1, 0, 2).reshape(128, G)
    prm[:, O_BDS:O_BDS + G] = sb1.transpose(1, 0, 2).reshape(128, G)
    prm[:, O_BPK:O_BPK + G] = kb2.transpose(1, 0, 2).reshape(128, G)
    prm[:, O_BPS:O_BPS + G] = sb2.transpose(1, 0, 2).reshape(128, G)

    sdiag = np.zeros((128, G * 9 * 128), np.float32)
    kdiag = np.zeros((128, G * 9 * 128), np.float32)
    idx = np.arange(128)
    for g in range(G):
        for t in range(9):
            o = (g * 9 + t) * 128
            sdiag[idx, o + idx] = sdw[g][:, t]
            kdiag[idx, o + idx] = kdw[g][:, t]

    prh = np.zeros((128, P_F16), np.float16)
    prh[:, O_WS:O_WS + G * G * 128] = spw.transpose(2, 0, 1, 3).reshape(
        128, G * G * 128).astype(np.float16)
    prh[:, O_WK:O_WK + G * G * 128] = kpw.transpose(2, 0, 1, 3).reshape(
        128, G * G * 128).astype(np.float16)
    prh[:, O_ID:O_ID + 128] = np.eye(128, dtype=np.float16)

    in_maps = []
    for i in range(N_CORES):
        sl = slice(i * BPC, (i + 1) * BPC)
        in_maps.append({"kern_in": np.ascontiguousarray(
                            kern[sl].transpose(1, 0, 2)),
                        "srch_in": srch[sl],
                        "params": prm, "sdiag": sdiag, "kdiag": kdiag,
                        "params16": prh})

    res = run_bass_kernel_spmd(nc, in_maps, list(range(N_CORES)))
    LAST_RESULTS = res
    out = np.concatenate([res.results[i]["out"] for i in range(N_CORES)], axis=0)
    return out.reshape(B, C, 25, 25)
